# revision 1
# baseline (speedup 1.0000x reference)
"""Trainium2 Bass kernel for the DCE 2CXM signal model — exp-Muentz basis.

Math: conc[k,n] = c1[n]*U[k,n] + c2[n]*V[k,n] with U[k,n] = F_k(theta_m[n]),
V[k,n] = F_k(theta_p[n]), where F_k(th) = sum_t A[k,t] exp(-0.1*t*th) is
approximated by a 64-term exponential sum F_k(th) ~= sum_j C[k,j]
exp(-0.1*s_j*th) fitted on the data's theta range (host, float64 ridge).
The c1/c2 normalization scalars collapse the he/hp normalizations
(geometric closed forms), as in the previous kernel.

The SPGR epilogue is rewritten via 1/(1-e^v) = 1/2 - coth(v/2)/2 so the
whole main loop needs only Exp + Tanh — both live in the same activation
table set (exp_and_others): zero table switches in the loop.

Device layout per core (12800 pixels):
 - prep in pixel-major [128, 100] (per-element ops cost free-size only),
 - rows [50, 512]: partition 2j = theta_m / c1 of subtile j, 2j+1 = theta_p / c2,
 - per 2-subtile iteration: PE K=2 ones-matmul broadcasts theta rows into
   PSUM [128, 1024] (rows 0-63 theta_m, 64-127 theta_p); ACT computes the
   exp basis with per-partition scale -0.1*s_j; PE broadcasts c1/c2 the same
   way; DVE+Pool scale the basis; PE contracts (K=64, accumulating the c1-
   and c2-scaled halves) into conc PSUM [100, 512]; ACT tanh; DVE recip;
   Pool affine; DMA out.
"""

import os
from contextlib import ExitStack

import numpy as np

H = W = 320
NPIX = H * W
NCORES = 8
SHARD = NPIX // NCORES      # 12800
NT = 512                    # pixels per subtile
NTILES = SHARD // NT        # 25
PC = 100                    # prep cols: [128, 100] pixel-major
TS = 50
KB = 64                     # basis size
STEP = 0.1
DELAY = 30
L = 589

SIG_BASELINE = 100.0
R1 = 1.0
R1CA = 4.3
FA = 10.0
TR = 0.00487

_CACHE: dict = {}


def _spgr_consts():
    f32 = np.float32
    fa = FA * np.pi / 180.0
    cosf = float(np.cos(f32(fa)))
    sinf = float(np.sin(f32(fa)))
    E1 = float(np.exp(f32(-TR * R1)))
    M0 = SIG_BASELINE * (1.0 - cosf * E1) / (sinf * (1.0 - E1))
    M0t = M0 * sinf
    M_st = M0t * (1.0 - E1) / (1.0 - E1 * cosf)
    C0 = SIG_BASELINE - M_st
    K1 = C0 + M0t / cosf
    K2 = M0t * (cosf - 1.0) / cosf
    K1p = K1 + K2 / 2.0
    K2p = -K2 / 2.0
    VH0 = 0.5 * (-TR * R1 + np.log(cosf))
    return K1p, K2p, VH0


def _build_bass():
    import concourse.bass as bass
    import concourse.tile as tile
    from concourse import bacc, mybir

    f32 = mybir.dt.float32
    f32r = mybir.dt.float32r
    AF = mybir.ActivationFunctionType
    ALU = mybir.AluOpType

    K1p, K2p, VH0 = _spgr_consts()

    nc = bacc.Bacc()
    pmap = nc.dram_tensor("pmap", [4, SHARD], f32, kind="ExternalInput")
    cmat = nc.dram_tensor("cmat", [KB, TS], f32, kind="ExternalInput")
    oblk = nc.dram_tensor("oblk", [2, 128], f32, kind="ExternalInput")
    svec = nc.dram_tensor("svec", [128, 1], f32, kind="ExternalInput")
    sig = nc.dram_tensor("sig", [TS, SHARD], f32, kind="ExternalOutput")

    with tile.TileContext(nc) as tc, ExitStack() as ctx:
        const = ctx.enter_context(tc.tile_pool(name="const", bufs=1))
        thps = ctx.enter_context(
            tc.tile_pool(name="thps", bufs=2, space=bass.MemorySpace.PSUM))
        cps = ctx.enter_context(
            tc.tile_pool(name="cps", bufs=1, space=bass.MemorySpace.PSUM))
        ccps = ctx.enter_context(
            tc.tile_pool(name="ccps", bufs=1, space=bass.MemorySpace.PSUM))
        baspool = ctx.enter_context(tc.tile_pool(name="bas", bufs=2))
        bspool = ctx.enter_context(tc.tile_pool(name="bs", bufs=2))
        epool = ctx.enter_context(tc.tile_pool(name="ep", bufs=2))
        opool = ctx.enter_context(tc.tile_pool(name="op", bufs=2))
        rows = ctx.enter_context(tc.tile_pool(name="rows", bufs=1))
        prep = ctx.enter_context(tc.tile_pool(name="prep", bufs=1))

        V = nc.vector
        G = nc.gpsimd

        # cmat duplicated at partitions 0:64 and 64:128 so lhsT base matches
        # whichever half of the basis the rhs slice starts at.
        cmat_sb = const.tile([128, TS], f32, tag="cmat_sb", name="cmat_sb")
        cmat_r = const.tile([128, TS], f32r, tag="cmat_r", name="cmat_r")
        oblk_sb = const.tile([2, 128], f32, tag="oblk_sb", name="oblk_sb")
        oblk_r = const.tile([2, 128], f32r, tag="oblk_r", name="oblk_r")
        sv_sb = const.tile([128, 1], f32, tag="sv_sb", name="sv_sb")
        b_vh = const.tile([128, 1], f32, tag="b_vh", name="b_vh")
        nc.sync.dma_start(out=cmat_sb[0:KB, :], in_=cmat[:])
        nc.sync.dma_start(out=cmat_sb[KB:128, :], in_=cmat[:])
        nc.sync.dma_start(out=oblk_sb, in_=oblk[:])
        nc.sync.dma_start(out=sv_sb, in_=svec[:])
        V.tensor_copy(cmat_r, cmat_sb)
        V.tensor_copy(oblk_r, oblk_sb)
        V.memset(b_vh, float(VH0))

        # ---------------- prep: pixel-major [128, 100] ----------------
        def pt(tag):
            return prep.tile([128, PC], f32, tag=tag, name=tag)

        ve, vp, fp, ps = (pt(t) for t in ("ve", "vp", "fp", "ps"))
        for i, t in enumerate((ve, vp, fp, ps)):
            nc.sync.dma_start(
                out=t, in_=pmap[i, :].rearrange("(p c) -> p c", p=128))

        rfp = pt("rfp"); V.reciprocal_approx_fast(rfp, fp)
        rps = pt("rps"); V.reciprocal_approx_fast(rps, ps)
        Te = pt("Te"); V.tensor_mul(Te, ve, rps)
        s_ = pt("s_"); G.tensor_add(s_, vp, ve)
        T_ = pt("T_"); V.tensor_mul(T_, s_, rfp)          # (vp+ve)/fp
        Tc = pt("Tc"); G.tensor_mul(Tc, vp, rfp)
        V.tensor_add(s_, T_, Te)                           # s = T+Te
        m4 = pt("m4")
        V.scalar_tensor_tensor(m4, Tc, 4.0, Te, op0=ALU.mult, op1=ALU.mult)
        sq = pt("sq"); V.tensor_mul(sq, s_, s_)
        V.tensor_sub(sq, sq, m4)
        disc = T_
        nc.scalar.sqrt(disc, sq)                           # SQRT table
        den = pt("den"); V.tensor_add(den, s_, disc)
        rden = pt("rden"); V.reciprocal_approx_fast(rden, den)
        thm = pt("thm"); V.tensor_scalar_mul(thm, rden, 2.0)
        rm4 = pt("rm4"); V.reciprocal_approx_fast(rm4, m4)
        thp = pt("thp")
        V.scalar_tensor_tensor(thp, den, 2.0, rm4, op0=ALU.mult, op1=ALU.mult)

        # geometric sums Sm, Sp  (EXP table; loop stays on this table)
        def geo(theta, tag):
            r1 = pt(tag + "_r1")
            nc.scalar.activation(r1, theta, AF.Exp, bias=0.0, scale=-STEP)
            rl = pt(tag + "_rl")
            nc.scalar.activation(rl, theta, AF.Exp, bias=0.0, scale=-STEP * L)
            V.tensor_scalar(rl, rl, -1.0, 1.0, op0=ALU.mult, op1=ALU.add)
            V.tensor_scalar(r1, r1, -1.0, 1.0, op0=ALU.mult, op1=ALU.add)
            V.reciprocal_approx_fast(r1, r1)
            V.tensor_mul(rl, rl, r1)
            return rl

        Sm = geo(thm, "gm")
        Sp = geo(thp, "gp")

        alp = pt("alp"); G.tensor_mul(alp, Te, thm)
        G.tensor_scalar(alp, alp, -1.0, 1.0, op0=ALU.mult, op1=ALU.add)
        bet = pt("bet"); G.tensor_mul(bet, Te, thp)
        G.tensor_scalar_sub(bet, bet, 1.0)

        de = pt("de"); V.tensor_sub(de, Sm, Sp)
        V.reciprocal_approx_fast(de, de)
        V.tensor_mul(Sm, alp, Sm)
        V.tensor_mul(Sp, bet, Sp)
        V.tensor_add(Sm, Sm, Sp)
        V.reciprocal_approx_fast(Sm, Sm)                   # 1/(a*Sm+b*Sp)
        u_ = rden
        V.tensor_mul(u_, ve, de)                           # ve/(Sm-Sp)
        V.tensor_mul(alp, alp, Sm)
        c1 = pt("c1"); V.tensor_mul(c1, vp, alp)
        V.tensor_add(c1, c1, u_)
        V.tensor_mul(bet, bet, Sm)
        c2 = pt("c2"); V.tensor_mul(c2, vp, bet)
        V.tensor_sub(c2, c2, u_)

        # rows: [2, SHARD] f32r (partition 0 = m-quantity, 1 = p-quantity);
        # f32r rounding happens in cheap pixel-major [128, 100] copies first.
        def to_rows(src_m, src_p, tag):
            t = rows.tile([2, SHARD], f32r, tag=tag, name=tag)
            for row, src in ((0, src_m), (1, src_p)):
                rsrc = prep.tile([128, PC], f32r, tag=tag + f"_rr{row}",
                                 name=tag + f"_rr{row}")
                V.tensor_copy(rsrc, src)
                nc.sync.dma_start(out=t[row:row + 1, :], in_=rsrc)
            return t

        rows_th = to_rows(thm, thp, "r_th")
        rows_c = to_rows(c1, c2, "r_c")

        # ---------------- main loop ----------------
        groups = []
        j = 0
        while j < NTILES:
            b = min(2, NTILES - j)
            groups.append((j, b))
            j += b
        ng = int(os.environ.get("DCE_NGROUPS", "0"))
        if ng:
            groups = groups[:ng]
        if os.environ.get("DCE_PREPONLY"):
            groups = []
        if groups or True:
            pass

        for j0, b in groups:
            Wg = b * NT
            STAGE = int(os.environ.get("DCE_STAGE", "8"))
            th_ps = thps.tile([128, 1024], f32, tag="th_ps", name="th_ps")
            for h in range(b):
                nc.tensor.matmul(
                    th_ps[:, h * NT:(h + 1) * NT], oblk_r,
                    rows_th[0:2, (j0 + h) * NT:(j0 + h + 1) * NT],
                    start=True, stop=True)
            if STAGE < 2:
                dbg = opool.tile([TS, 1024], f32, tag="dbg", name="dbg")
                V.tensor_copy(dbg[:, :Wg], th_ps[0:TS, :Wg])
                nc.sync.dma_start(out=sig[:, j0 * NT:j0 * NT + Wg], in_=dbg[:, :Wg])
                continue
            bas = baspool.tile([128, 1024], f32r, tag="bas", name="bas")
            nc.scalar.activation(
                bas[:, :Wg], th_ps[:, :Wg], AF.Exp,
                bias=0.0, scale=sv_sb[:, 0:1])

            if STAGE < 3:
                dbg = opool.tile([TS, 1024], f32, tag="dbg", name="dbg")
                V.tensor_copy(dbg[:, :Wg], bas[0:TS, :Wg].bitcast(f32))
                nc.sync.dma_start(out=sig[:, j0 * NT:j0 * NT + Wg], in_=dbg[:, :Wg])
                continue
            c_ps = cps.tile([128, 1024], f32, tag="c_ps", name="c_ps")
            for h in range(b):
                nc.tensor.matmul(
                    c_ps[:, h * NT:(h + 1) * NT], oblk_r,
                    rows_c[0:2, (j0 + h) * NT:(j0 + h + 1) * NT],
                    start=True, stop=True)

            if STAGE < 4:
                dbg = opool.tile([TS, 1024], f32, tag="dbg", name="dbg")
                V.tensor_copy(dbg[:, :Wg], c_ps[0:TS, :Wg])
                nc.sync.dma_start(out=sig[:, j0 * NT:j0 * NT + Wg], in_=dbg[:, :Wg])
                continue
            bs = bspool.tile([128, 1024], f32r, tag="bs", name="bs")
            V.tensor_mul(bs[:, :Wg], bas[:, :Wg].bitcast(f32), c_ps[:, :Wg])

            if STAGE < 5:
                dbg = opool.tile([TS, 1024], f32, tag="dbg", name="dbg")
                V.tensor_copy(dbg[:, :Wg], bs[0:TS, :Wg].bitcast(f32))
                nc.sync.dma_start(out=sig[:, j0 * NT:j0 * NT + Wg], in_=dbg[:, :Wg])
                continue
            # conc column-stacked: subtile A at cols 0:512, B at 512:1024
            conc_ps = ccps.tile([TS, 1024], f32, tag="conc_ps", name="conc_ps")
            for h in range(b):
                lo = h * NT
                nc.tensor.matmul(conc_ps[:, lo:lo + NT], cmat_r,
                                 bs[:, lo:lo + NT], start=True, stop=True)

            if STAGE < 6:
                dbg = opool.tile([TS, 1024], f32, tag="dbg", name="dbg")
                V.tensor_copy(dbg[:, :Wg], conc_ps[:, :Wg])
                nc.sync.dma_start(out=sig[:, j0 * NT:j0 * NT + Wg], in_=dbg[:, :Wg])
                continue
            th_t = epool.tile([TS, 1024], f32, tag="th_t", name="th_t")
            nc.scalar.activation(
                th_t[:, :Wg], conc_ps[:, :Wg], AF.Tanh,
                bias=b_vh[0:TS, 0:1], scale=float(-TR * R1CA / 2.0))
            rt = epool.tile([TS, 1024], f32, tag="rt", name="rt")
            V.reciprocal_approx_fast(rt[:, :Wg], th_t[:, :Wg])
            out_t = opool.tile([TS, 1024], f32, tag="out_t", name="out_t")
            G.tensor_scalar(out_t[:, :Wg], rt[:, :Wg], float(K2p), float(K1p),
                            op0=ALU.mult, op1=ALU.add)
            nc.sync.dma_start(out=sig[:, j0 * NT:j0 * NT + Wg],
                              in_=out_t[:, :Wg])

    nc.compile()
    return nc


def _host_prep(sample_time: np.ndarray, Cp: np.ndarray):
    t_end = float(np.asarray(sample_time)[-1])
    Lf = int(round(t_end / STEP)) + 1
    t_samp = np.arange(Lf, dtype=np.float32) * np.float32(STEP)
    aifci = np.interp(
        t_samp.astype(np.float64),
        np.asarray(sample_time, np.float64),
        np.asarray(Cp, np.float64),
    ).astype(np.float32)
    aif = np.concatenate([np.zeros(DELAY, np.float32), aifci[:-DELAY]])
    idx = np.searchsorted(t_samp, np.asarray(sample_time, np.float32), side="left")
    idx = np.minimum(idx, Lf - 1)
    A = np.zeros((TS, Lf), np.float64)
    for k in range(TS):
        i = int(idx[k])
        A[k, : i + 1] = aif[i::-1]

    sj = np.concatenate([[0.0], np.geomspace(0.15, 588.0, KB - 1)])
    th_grid = np.geomspace(0.012, 70.0, 6000)
    E = np.exp(-STEP * np.outer(th_grid, np.arange(Lf)))
    F = E @ A.T
    B = np.exp(-STEP * np.outer(th_grid, sj))
    lam = 1e-9 * np.linalg.norm(B, 2) ** 2
    C = np.linalg.solve(B.T @ B + lam * np.eye(KB), B.T @ F).T   # [TS, KB]

    cmat = np.ascontiguousarray(C.T.astype(np.float32))          # [KB, TS]
    oblk = np.zeros((2, 128), np.float32)
    oblk[0, 0:64] = 1.0
    oblk[1, 64:128] = 1.0
    svec = (-STEP * sj[(np.arange(128) % KB)]).astype(np.float32).reshape(128, 1)
    return cmat, oblk, svec


def kernel(param: np.ndarray, sample_time: np.ndarray, Cp: np.ndarray) -> np.ndarray:
    from concourse.bass_utils import run_bass_kernel_spmd

    if "nc" not in _CACHE:
        _CACHE["nc"] = _build_bass()
    nc = _CACHE["nc"]

    cmat, oblk, svec = _host_prep(sample_time, Cp)
    pflat = np.ascontiguousarray(np.asarray(param, np.float32).reshape(4, NPIX))
    in_maps = []
    for c in range(NCORES):
        in_maps.append({
            "pmap": np.ascontiguousarray(pflat[:, c * SHARD:(c + 1) * SHARD]),
            "cmat": cmat, "oblk": oblk, "svec": svec,
        })
    ncr = int(os.environ.get("DCE_CORES", str(NCORES)))
    res = run_bass_kernel_spmd(
        nc, in_maps[:ncr], core_ids=list(range(ncr)),
        trace=bool(int(os.environ.get("DCE_TRACE", "0"))),
    )
    if res.exec_time_ns is not None:
        _CACHE["exec_time_ns"] = res.exec_time_ns
    outs = [r["sig"] for r in res.results]
    while len(outs) < NCORES:
        outs.append(np.zeros((TS, SHARD), np.float32))
    out = np.concatenate(outs, axis=1)
    return out.reshape(TS, 1, H, W)



# revision 14
# speedup vs baseline: 1.2484x; 1.2484x over previous
"""Trainium2 Bass kernel for the DCE 2CXM signal model — log-folded 4-channel
exp basis, reciprocal epilogue.

Math per pixel: theta_m/theta_p from the 2CXM params; conc[k] =
a1*Gk(thm) + a2*Gk(thp) + u*(Gk(thm) - Gk(thp)) with a1, a2, u all > 0
(a1 = vp*alpha/(alpha*Sm+beta*Sp), a2 = vp*beta/(...), u = ve/(Sm-Sp)).
Gk(th) = sum_t A[k,t] exp(-0.1 t th) is fitted per channel with 16 (or 15)
exponential nodes on the channel's empirical theta range.  The positive
coefficients are folded into the exponentials via logs:
c*exp(-0.1 s th) = exp(-0.1 s th + ln c), so one broadcast matmul (mm1,
K=10) builds all 128 exp arguments per column (2 pixels/column, 64
partitions each: 16 a1|m + 16 a2|p + 16 u|m + 15 u|p + 1 zero -> exp=1),
one ACT Exp evaluates the basis, and one block-diagonal matmul (mm2,
K=128 -> M=100) contracts straight to P' = (VH0 + s*conc)/K2p for both
pixel halves at once (the exp(0)=1 row carries the VH0 bias).  The SPGR
epilogue uses coth(v) ~= 1/v (abs err < 0.04 on a ~300 signal):
sig = K1p + K2p/v, i.e. one DVE reciprocal + one add (+fp16 cast) spread
over ACT/Pool.  Output leaves the device in fp16; host adds K1p during
the fp32 upcast.
"""

import os
from contextlib import ExitStack

import numpy as np

H = W = 320
NPIX = H * W
NCORES = 8
SHARD = NPIX // NCORES      # 12800 pixels per core
HALF = SHARD // 2           # 6400   (2 pixels per basis column)
PC = 100                    # prep layout [128, 100]
TS = 50
STEP = 0.1
DELAY = 30
LF = 589
SC = 1024                   # superchunk columns

SIG_BASELINE = 100.0
R1 = 1.0
R1CA = 4.3
FA = 10.0
TR = 0.00487

_CACHE: dict = {}


def _spgr_consts():
    f32 = np.float32
    fa = FA * np.pi / 180.0
    cosf = float(np.cos(f32(fa)))
    sinf = float(np.sin(f32(fa)))
    E1 = float(np.exp(f32(-TR * R1)))
    M0 = SIG_BASELINE * (1.0 - cosf * E1) / (sinf * (1.0 - E1))
    M0t = M0 * sinf
    M_st = M0t * (1.0 - E1) / (1.0 - E1 * cosf)
    C0 = SIG_BASELINE - M_st
    K1 = C0 + M0t / cosf
    K2 = M0t * (cosf - 1.0) / cosf
    K1p = K1 + K2 / 2.0
    K2p = -K2 / 2.0
    VH0 = 0.5 * (-TR * R1 + np.log(cosf))
    SS = -TR * R1CA / 2.0
    return K1p, K2p, VH0, SS


def _build_bass():
    import concourse.bass as bass
    import concourse.tile as tile
    from concourse import bacc, mybir

    f32 = mybir.dt.float32
    f32r = mybir.dt.float32r
    f16 = mybir.dt.float16
    AF = mybir.ActivationFunctionType
    ALU = mybir.AluOpType

    nc = bacc.Bacc()
    pmap = nc.dram_tensor("pmap", [4, SHARD], f32, kind="ExternalInput")
    w1 = nc.dram_tensor("w1", [10, 128], f32, kind="ExternalInput")
    w2 = nc.dram_tensor("w2", [128, PC], f32, kind="ExternalInput")
    sig = nc.dram_tensor("sig", [TS, SHARD], f16, kind="ExternalOutput")

    K1p, K2p, VH0, SS = _spgr_consts()
    NWARM = int(os.environ.get("DCE_WARM", "6"))

    with tile.TileContext(nc) as tc, ExitStack() as ctx:
        const = ctx.enter_context(tc.tile_pool(name="const", bufs=1))
        thps = ctx.enter_context(
            tc.tile_pool(name="thps", bufs=2, space=bass.MemorySpace.PSUM))
        pps = ctx.enter_context(
            tc.tile_pool(name="pps", bufs=2, space=bass.MemorySpace.PSUM))
        baspool = ctx.enter_context(tc.tile_pool(name="bas", bufs=2))
        rpool = ctx.enter_context(tc.tile_pool(name="rp", bufs=2))
        opool = ctx.enter_context(tc.tile_pool(name="op", bufs=2))
        rows = ctx.enter_context(tc.tile_pool(name="rows", bufs=1))
        prep = ctx.enter_context(tc.tile_pool(name="prep", bufs=1))

        V = nc.vector
        G = nc.gpsimd
        SCL = nc.scalar

        # Tiny memset+exp first: starts the (single) ACT table load for
        # natural_log_exp at t~0, overlapped with the input DMAs.
        tl = const.tile([1, 1], f32, tag="tl", name="tl")
        V.memset(tl, 0.0)
        SCL.activation(tl, tl, AF.Exp, bias=0.0, scale=1.0)

        lhsT1_t = const.tile([10, 128], f32, tag="lhsT1f", name="lhsT1f")
        lhsT2_t = const.tile([128, PC], f32, tag="lhsT2f", name="lhsT2f")
        nc.sync.dma_start(out=lhsT1_t, in_=w1[:])
        nc.sync.dma_start(out=lhsT2_t, in_=w2[:])
        lhsT1 = const.tile([10, 128], f32r, tag="lhsT1", name="lhsT1")
        lhsT2 = const.tile([128, PC], f32r, tag="lhsT2", name="lhsT2")
        V.tensor_copy(lhsT1, lhsT1_t)
        V.tensor_copy(lhsT2, lhsT2_t)

        # PE warm-up: garbage matmuls during prep so the HAM ramp (~3us)
        # completes before the real main-loop matmuls.
        for wi in range(NWARM):
            wt = thps.tile([128, SC], f32, tag="th_ps", name=f"warm{wi}")
            nc.tensor.matmul(wt[0:PC, 0:PC], lhsT2, lhsT2[:, 0:PC],
                             start=True, stop=True)

        # ---------------- prep: pixel-major [128, 100] ----------------
        def pt(tag, dt=f32):
            return prep.tile([128, PC], dt, tag=tag, name=tag)

        ve, vp, fp_, ps_ = (pt(t) for t in ("ve", "vp", "fp", "ps"))
        for i, t in enumerate((ve, vp, fp_, ps_)):
            nc.sync.dma_start(
                out=t, in_=pmap[i, :].rearrange("(p c) -> p c", p=128))

        out5 = prep.tile([128, 5 * PC], f32r, tag="out5", name="out5")
        thm_h = out5[:, 0 * PC:1 * PC]
        thp_h = out5[:, 1 * PC:2 * PC]
        lna1 = out5[:, 2 * PC:3 * PC]
        lna2 = out5[:, 3 * PC:4 * PC]
        lnu = out5[:, 4 * PC:5 * PC]

        rfp = pt("rfp"); V.reciprocal_approx_fast(rfp, fp_)
        rps = pt("rps"); V.reciprocal_approx_fast(rps, ps_)
        Te = pt("Te"); G.tensor_mul(Te, ve, rps)
        svp = pt("svp"); G.tensor_add(svp, vp, ve)
        T_ = pt("T_"); V.tensor_mul(T_, svp, rfp)
        Tc = pt("Tc"); G.tensor_mul(Tc, vp, rfp)
        S_ = pt("S_"); V.tensor_add(S_, T_, Te)
        TcTe = pt("TcTe"); G.tensor_mul(TcTe, Tc, Te)
        S2 = pt("S2")
        V.scalar_tensor_tensor(S2, S_, 1.0, S_, op0=ALU.mult, op1=ALU.mult)
        m4 = pt("m4"); G.tensor_scalar_mul(m4, TcTe, 4.0)
        d2 = pt("d2"); G.tensor_sub(d2, S2, m4)
        lnd = pt("lnd"); SCL.activation(lnd, d2, AF.Ln, bias=0.0, scale=1.0)
        disc = pt("disc")
        SCL.activation(disc, lnd, AF.Exp, bias=0.0, scale=0.5)
        den_ = pt("den"); G.tensor_add(den_, S_, disc)
        thmt = pt("thmt"); V.reciprocal_approx_fast(thmt, den_)
        G.tensor_copy(thm_h, thmt)
        rTT = pt("rTT"); V.reciprocal_approx_fast(rTT, TcTe)
        G.tensor_mul(thp_h, den_, rTT)

        thm_r = thm_h.bitcast(f32)
        thp_r = thp_h.bitcast(f32)
        r1m = pt("r1m")
        SCL.activation(r1m, thm_r, AF.Exp, bias=0.0, scale=-0.2)
        rlm = pt("rlm")
        SCL.activation(rlm, thm_r, AF.Exp, bias=0.0, scale=-0.2 * LF)
        r1p = pt("r1p")
        SCL.activation(r1p, thp_r, AF.Exp, bias=0.0, scale=-0.05)

        Dm = pt("Dm"); V.tensor_scalar(Dm, r1m, -1.0, 1.0, op0=ALU.mult, op1=ALU.add)
        Dp = pt("Dp"); G.tensor_scalar(Dp, r1p, -1.0, 1.0, op0=ALU.mult, op1=ALU.add)
        Nm = pt("Nm"); V.tensor_scalar(Nm, rlm, -1.0, 1.0, op0=ALU.mult, op1=ALU.add)
        P1 = pt("P1"); V.tensor_mul(P1, Nm, Dp)
        W_ = pt("W_"); G.tensor_mul(W_, Dm, Dp)
        alt = pt("alt")
        V.scalar_tensor_tensor(alt, Te, -2.0, thm_r, op0=ALU.mult, op1=ALU.mult)
        al = pt("al"); G.tensor_scalar_add(al, alt, 1.0)
        btt = pt("btt")
        V.scalar_tensor_tensor(btt, Te, 0.5, thp_r, op0=ALU.mult, op1=ALU.mult)
        bt = pt("bt"); G.tensor_scalar_sub(bt, btt, 1.0)
        aP1 = pt("aP1"); V.tensor_mul(aP1, al, P1)
        bP2 = pt("bP2"); G.tensor_mul(bP2, bt, Dm)
        den1 = pt("den1"); V.tensor_add(den1, aP1, bP2)
        dd = pt("dd"); G.tensor_sub(dd, P1, Dm)
        r1_ = pt("r1_"); V.reciprocal_approx_fast(r1_, den1)
        rdd = pt("rdd"); V.reciprocal_approx_fast(rdd, dd)
        vpW = pt("vpW"); G.tensor_mul(vpW, vp, W_)
        veW = pt("veW"); V.tensor_mul(veW, ve, W_)
        t4 = pt("t4"); G.tensor_mul(t4, vpW, al)
        a1t = pt("a1t"); V.tensor_mul(a1t, t4, r1_)
        SCL.activation(lna1, a1t, AF.Ln, bias=0.0, scale=1.0)
        t5 = pt("t5"); V.tensor_mul(t5, vpW, bt)
        a2t = pt("a2t"); G.tensor_mul(a2t, t5, r1_)
        SCL.activation(lna2, a2t, AF.Ln, bias=0.0, scale=1.0)
        ut = pt("ut"); G.tensor_mul(ut, veW, rdd)
        SCL.activation(lnu, ut, AF.Ln, bias=0.0, scale=1.0)

        # rows [10, HALF]: row h*5+v <- out5[h*64+s, v*100+c] at col s*100+c
        rows_t = rows.tile([10, HALF], f32r, tag="rows", name="rows")
        for h in range(2):
            for v in range(5):
                nc.sync.dma_start(
                    out=rows_t[h * 5 + v: h * 5 + v + 1, :],
                    in_=out5[h * 64:(h + 1) * 64, v * PC:(v + 1) * PC])

        # ---------------- main loop ----------------
        n_sc = (HALF + SC - 1) // SC     # 7 (6x1024 + 1x256)
        for isc in range(n_sc):
            base = isc * SC
            cols = min(SC, HALF - base)
            th_ps = thps.tile([128, SC], f32, tag="th_ps", name=f"th_ps{isc}")
            for o in range(0, cols, 512):
                w = min(512, cols - o)
                nc.tensor.matmul(
                    th_ps[:, o:o + w], lhsT1,
                    rows_t[0:10, base + o:base + o + w],
                    start=True, stop=True)
            bas = baspool.tile([128, SC], f32r, tag="bas", name=f"bas{isc}")
            SCL.activation(bas[:, :cols], th_ps[:, :cols], AF.Exp,
                           bias=0.0, scale=1.0)
            p_ps = pps.tile([100, SC], f32, tag="p_ps", name=f"p_ps{isc}")
            for o in range(0, cols, 512):
                w = min(512, cols - o)
                nc.tensor.matmul(
                    p_ps[:, o:o + w], lhsT2,
                    bas[:, o:o + w],
                    start=True, stop=True)
            rt = rpool.tile([100, SC], f32, tag="rt", name=f"rt{isc}")
            V.reciprocal_approx_fast(rt[:, :cols], p_ps[:, :cols])
            out_t = opool.tile([100, SC], f16, tag="out_t", name=f"out_t{isc}")
            if isc % 4 == 0:
                SCL.activation(out_t[:, :cols], rt[:, :cols], AF.Copy,
                               bias=float(K1p), scale=1.0)
            else:
                G.tensor_scalar_add(out_t[:, :cols], rt[:, :cols], float(K1p))
            for h in range(2):
                nc.sync.dma_start(
                    out=sig[:, h * HALF + base: h * HALF + base + cols],
                    in_=out_t[h * TS:(h + 1) * TS, :cols])

    nc.compile()
    return nc


def _host_prep(param: np.ndarray, sample_time: np.ndarray, Cp: np.ndarray):
    """AIF conv matrix + per-channel exponential-sum fits -> lhsT1/lhsT2."""
    f32 = np.float32
    t32 = np.arange(LF, dtype=f32) * f32(STEP)
    aifci = np.interp(
        t32.astype(np.float64),
        np.asarray(sample_time, np.float64),
        np.asarray(Cp, np.float64))
    aif = np.concatenate([np.zeros(DELAY), aifci[:-DELAY]])
    idx = np.minimum(
        np.searchsorted(t32, np.asarray(sample_time, f32), side="left"),
        LF - 1)
    A = np.zeros((TS, LF))
    for k in range(TS):
        i = int(idx[k])
        A[k, : i + 1] = aif[i::-1]

    # empirical theta ranges (cheap fp64 host pass over the param maps)
    ve, vp, fp_, ps_ = [np.asarray(param[i], np.float64).ravel()
                        for i in range(4)]
    Te = ve / ps_
    S = (vp + ve) / fp_ + Te
    TcTe = (vp / fp_) * Te
    disc = np.sqrt(S * S - 4.0 * TcTe)
    thm = 2.0 / (S + disc)
    thp = (S + disc) / (2.0 * TcTe)
    tm = (float(thm.min()) * 0.98, float(thm.max()) * 1.02)
    tp = (float(thp.min()) * 0.98, float(thp.max()) * 1.02)

    def fit(nodes, tlo, thi, ngrid=3000, lam_rel=1e-10):
        th = np.geomspace(tlo, thi, ngrid)
        Eg = np.exp(-STEP * np.outer(th, np.arange(LF)))
        F = Eg @ A.T
        B = np.exp(-STEP * np.outer(th, nodes))
        lam = lam_rel * np.linalg.norm(B, 2) ** 2
        return np.linalg.solve(B.T @ B + lam * np.eye(len(nodes)), B.T @ F)

    nm16 = np.concatenate([[0.0], np.geomspace(0.5, 588.0, 15)])
    np16 = np.concatenate([[0.0], np.geomspace(0.1, 588.0, 15)])
    np15 = np.concatenate([[0.0], np.geomspace(0.1, 588.0, 14)])
    Cm = fit(nm16, *tm)      # [16, TS]
    Cq = fit(np16, *tp)      # [16, TS]
    Cq15 = fit(np15, *tp)    # [15, TS]

    K1p, K2p, VH0, SS = _spgr_consts()
    q = SS / K2p

    w2 = np.zeros((128, PC), f32)
    w1 = np.zeros((10, 128), f32)
    for h in range(2):
        b = h * 64
        k0 = h * TS
        w2[b + 0:b + 16, k0:k0 + TS] = (q * Cm).astype(f32)
        w2[b + 16:b + 32, k0:k0 + TS] = (q * Cq).astype(f32)
        w2[b + 32:b + 48, k0:k0 + TS] = (q * Cm).astype(f32)
        w2[b + 48:b + 63, k0:k0 + TS] = (-q * Cq15).astype(f32)
        w2[b + 63, k0:k0 + TS] = f32(VH0 / K2p)
        r0 = h * 5
        w1[r0 + 0, b + 0:b + 16] = (-0.2 * nm16).astype(f32)
        w1[r0 + 0, b + 32:b + 48] = (-0.2 * nm16).astype(f32)
        w1[r0 + 1, b + 16:b + 32] = (-0.05 * np16).astype(f32)
        w1[r0 + 1, b + 48:b + 63] = (-0.05 * np15).astype(f32)
        w1[r0 + 2, b + 0:b + 16] = 1.0
        w1[r0 + 3, b + 16:b + 32] = 1.0
        w1[r0 + 4, b + 32:b + 63] = 1.0
    return w1, w2, f32(K1p)


def kernel(param: np.ndarray, sample_time: np.ndarray, Cp: np.ndarray) -> np.ndarray:
    from concourse.bass_utils import run_bass_kernel_spmd

    if "nc" not in _CACHE:
        _CACHE["nc"] = _build_bass()
    nc = _CACHE["nc"]

    w1, w2, K1p = _host_prep(param, sample_time, Cp)
    pflat = np.ascontiguousarray(np.asarray(param, np.float32).reshape(4, NPIX))
    in_maps = []
    for c in range(NCORES):
        in_maps.append({
            "pmap": np.ascontiguousarray(pflat[:, c * SHARD:(c + 1) * SHARD]),
            "w1": w1, "w2": w2,
        })
    ncr = int(os.environ.get("DCE_CORES", str(NCORES)))
    res = run_bass_kernel_spmd(
        nc, in_maps[:ncr], core_ids=list(range(ncr)),
        trace=bool(int(os.environ.get("DCE_TRACE", "0"))),
    )
    if res.exec_time_ns is not None:
        _CACHE["exec_time_ns"] = res.exec_time_ns
    outs = [r["sig"] for r in res.results]
    while len(outs) < NCORES:
        outs.append(np.zeros((TS, SHARD), np.float16))
    out = np.concatenate(outs, axis=1).astype(np.float32)
    return out.reshape(TS, 1, H, W)


# revision 29
# speedup vs baseline: 1.4719x; 1.1791x over previous
"""Trainium2 Bass kernel for the DCE 2CXM signal model — log-folded 4-channel
exp basis, reciprocal epilogue.

Math per pixel: theta_m/theta_p from the 2CXM params; conc[k] =
a1*Gk(thm) + a2*Gk(thp) + u*(Gk(thm) - Gk(thp)) with a1, a2, u all > 0
(a1 = vp*alpha/(alpha*Sm+beta*Sp), a2 = vp*beta/(...), u = ve/(Sm-Sp)).
Gk(th) = sum_t A[k,t] exp(-0.1 t th) is fitted per channel with 16 (or 15)
exponential nodes on the channel's empirical theta range.  The positive
coefficients are folded into the exponentials via logs:
c*exp(-0.1 s th) = exp(-0.1 s th + ln c), so one broadcast matmul (mm1,
K=10) builds all 128 exp arguments per column (2 pixels/column, 64
partitions each: 16 a1|m + 16 a2|p + 16 u|m + 15 u|p + 1 zero -> exp=1),
one ACT Exp evaluates the basis, and one block-diagonal matmul (mm2,
K=128 -> M=100) contracts straight to P' = (VH0 + s*conc)/K2p for both
pixel halves at once (the exp(0)=1 row carries the VH0 bias).  The SPGR
epilogue uses coth(v) ~= 1/v (abs err < 0.04 on a ~300 signal):
sig = K1p + K2p/v, i.e. one DVE reciprocal + one add (+fp16 cast) spread
over ACT/Pool.  Output leaves the device in fp16; host adds K1p during
the fp32 upcast.
"""

import os
from contextlib import ExitStack

import numpy as np

H = W = 320
NPIX = H * W
NCORES = 8
SHARD = NPIX // NCORES      # 12800 pixels per core
HALF = SHARD // 2           # 6400   (2 pixels per basis column)
PC = 100                    # prep layout [128, 100]
TS = 50
STEP = 0.1
DELAY = 30
LF = 589
SC = 1024                   # superchunk columns

SIG_BASELINE = 100.0
R1 = 1.0
R1CA = 4.3
FA = 10.0
TR = 0.00487

_CACHE: dict = {}


def _spgr_consts():
    f32 = np.float32
    fa = FA * np.pi / 180.0
    cosf = float(np.cos(f32(fa)))
    sinf = float(np.sin(f32(fa)))
    E1 = float(np.exp(f32(-TR * R1)))
    M0 = SIG_BASELINE * (1.0 - cosf * E1) / (sinf * (1.0 - E1))
    M0t = M0 * sinf
    M_st = M0t * (1.0 - E1) / (1.0 - E1 * cosf)
    C0 = SIG_BASELINE - M_st
    K1 = C0 + M0t / cosf
    K2 = M0t * (cosf - 1.0) / cosf
    K1p = K1 + K2 / 2.0
    K2p = -K2 / 2.0
    VH0 = 0.5 * (-TR * R1 + np.log(cosf))
    SS = -TR * R1CA / 2.0
    return K1p, K2p, VH0, SS


def _patch_act_tables():
    """Make Exp/Ln/Copy resolve only to natural_log_exp_and_others so the
    table-load pass emits a single load instead of ping-ponging between
    exp_and_others and natural_log_exp_and_others (1.3us per switch)."""
    import concourse.bacc as bacc_mod
    from concourse import mybir
    from concourse.hw_specs import get_activation_tables as _orig

    AF = mybir.ActivationFunctionType
    mine = {AF.Exp, AF.Ln, AF.Copy, AF.Identity}

    def patched(arch):
        tabs = _orig(arch)
        out = {}
        for name, fns in tabs.items():
            if name == "natural_log_exp_and_others":
                out[name] = set(fns) | {AF.Copy, AF.Identity}
            else:
                out[name] = set(fns) - mine
        return out

    bacc_mod.get_activation_tables = patched


def _build_bass():
    import concourse.bass as bass
    import concourse.tile as tile
    from concourse import bacc, mybir

    _patch_act_tables()

    f32 = mybir.dt.float32
    f32r = mybir.dt.float32r
    f16 = mybir.dt.float16
    AF = mybir.ActivationFunctionType
    ALU = mybir.AluOpType

    nc = bacc.Bacc()
    pmap = nc.dram_tensor("pmap", [4, SHARD], f32, kind="ExternalInput")
    wall = nc.dram_tensor("wall", [128, PC + 128], f32, kind="ExternalInput")
    sig = nc.dram_tensor("sig", [TS, SHARD], f16, kind="ExternalOutput")

    K1p, K2p, VH0, SS = _spgr_consts()
    NWARM = int(os.environ.get("DCE_WARM", "6"))

    with tile.TileContext(nc) as tc, ExitStack() as ctx:
        const = ctx.enter_context(tc.tile_pool(name="const", bufs=1))
        thps = ctx.enter_context(
            tc.tile_pool(name="thps", bufs=2, space=bass.MemorySpace.PSUM))
        pps = ctx.enter_context(
            tc.tile_pool(name="pps", bufs=2, space=bass.MemorySpace.PSUM))
        baspool = ctx.enter_context(tc.tile_pool(name="bas", bufs=2))
        rpool = ctx.enter_context(tc.tile_pool(name="rp", bufs=3))
        opool = ctx.enter_context(tc.tile_pool(name="op", bufs=3))
        rows = ctx.enter_context(tc.tile_pool(name="rows", bufs=1))
        prep = ctx.enter_context(tc.tile_pool(name="prep", bufs=1))

        V = nc.vector
        G = nc.gpsimd
        SCL = nc.scalar

        # Tiny memset+exp first: starts the (single) ACT table load for
        # natural_log_exp at t~0, overlapped with the input DMAs.
        tl = const.tile([1, 1], f32, tag="tl", name="tl")
        V.memset(tl, 0.0)
        SCL.activation(tl, tl, AF.Exp, bias=0.0, scale=1.0)

        wtile_f = const.tile([128, PC + 128], f32, tag="wallf", name="wallf")
        nc.sync.dma_start(out=wtile_f, in_=wall[:])
        wtile = const.tile([128, PC + 128], f32r, tag="wallr", name="wallr")
        V.tensor_copy(wtile, wtile_f)
        lhsT2 = wtile[:, 0:PC]
        lhsT1 = wtile[0:10, PC:PC + 128]

        # PE warm-up: garbage matmuls during prep so the HAM ramp (~3us)
        # completes before the real main-loop matmuls.
        for wi in range(NWARM):
            wt = thps.tile([128, SC], f32, tag="th_ps", name=f"warm{wi}")
            nc.tensor.matmul(wt[0:PC, 0:PC], lhsT2, lhsT2[:, 0:PC],
                             start=True, stop=True)

        # ---------------- prep: pixel-major [128, 100] ----------------
        def pt(tag, dt=f32):
            return prep.tile([128, PC], dt, tag=tag, name=tag)

        pin = prep.tile([128, 4 * PC], f32, tag="pin", name="pin")
        nc.sync.dma_start(
            out=pin[:].rearrange("p (i c) -> p i c", i=4),
            in_=pmap[:].rearrange("i (p c) -> p i c", p=128))
        ve = pin[:, 0 * PC:1 * PC]
        vp = pin[:, 1 * PC:2 * PC]
        fp_ = pin[:, 2 * PC:3 * PC]
        ps_ = pin[:, 3 * PC:4 * PC]

        out5 = prep.tile([128, 5 * PC], f32r, tag="out5", name="out5")
        thm_h = out5[:, 0 * PC:1 * PC]
        thp_h = out5[:, 1 * PC:2 * PC]
        lna1 = out5[:, 2 * PC:3 * PC]
        lna2 = out5[:, 3 * PC:4 * PC]
        lnu = out5[:, 4 * PC:5 * PC]

        rfp = pt("rfp"); V.reciprocal_approx_fast(rfp, fp_)
        rps = pt("rps"); V.reciprocal_approx_fast(rps, ps_)
        Te = pt("Te"); G.tensor_mul(Te, ve, rps)
        svp = pt("svp"); G.tensor_add(svp, vp, ve)
        T_ = pt("T_"); V.tensor_mul(T_, svp, rfp)
        Tc = pt("Tc"); G.tensor_mul(Tc, vp, rfp)
        S_ = pt("S_"); V.tensor_add(S_, T_, Te)
        TcTe = pt("TcTe"); G.tensor_mul(TcTe, Tc, Te)
        S2 = pt("S2")
        V.scalar_tensor_tensor(S2, S_, 1.0, S_, op0=ALU.mult, op1=ALU.mult)
        m4 = pt("m4"); G.tensor_scalar_mul(m4, TcTe, 4.0)
        d2 = pt("d2"); G.tensor_sub(d2, S2, m4)
        lnd = pt("lnd"); SCL.activation(lnd, d2, AF.Ln, bias=0.0, scale=1.0)
        disc = pt("disc")
        SCL.activation(disc, lnd, AF.Exp, bias=0.0, scale=0.5)
        den_ = pt("den"); G.tensor_add(den_, S_, disc)
        thmt = pt("thmt"); V.reciprocal_approx_fast(thmt, den_)
        G.tensor_copy(thm_h, thmt)
        rTT = pt("rTT"); V.reciprocal_approx_fast(rTT, TcTe)
        G.tensor_mul(thp_h, den_, rTT)

        thm_r = thm_h.bitcast(f32)
        thp_r = thp_h.bitcast(f32)
        r1m = pt("r1m")
        SCL.activation(r1m, thm_r, AF.Exp, bias=0.0, scale=-0.2)
        rlm = pt("rlm")
        SCL.activation(rlm, thm_r, AF.Exp, bias=0.0, scale=-0.2 * LF)
        r1p = pt("r1p")
        SCL.activation(r1p, thp_r, AF.Exp, bias=0.0, scale=-0.05)

        Dm = pt("Dm"); V.tensor_scalar(Dm, r1m, -1.0, 1.0, op0=ALU.mult, op1=ALU.add)
        Dp = pt("Dp"); G.tensor_scalar(Dp, r1p, -1.0, 1.0, op0=ALU.mult, op1=ALU.add)
        Nm = pt("Nm"); V.tensor_scalar(Nm, rlm, -1.0, 1.0, op0=ALU.mult, op1=ALU.add)
        P1 = pt("P1"); V.tensor_mul(P1, Nm, Dp)
        W_ = pt("W_"); G.tensor_mul(W_, Dm, Dp)
        alt = pt("alt")
        V.scalar_tensor_tensor(alt, Te, -2.0, thm_r, op0=ALU.mult, op1=ALU.mult)
        al = pt("al"); G.tensor_scalar_add(al, alt, 1.0)
        btt = pt("btt")
        V.scalar_tensor_tensor(btt, Te, 0.5, thp_r, op0=ALU.mult, op1=ALU.mult)
        bt = pt("bt"); G.tensor_scalar_sub(bt, btt, 1.0)
        aP1 = pt("aP1"); V.tensor_mul(aP1, al, P1)
        bP2 = pt("bP2"); G.tensor_mul(bP2, bt, Dm)
        den1 = pt("den1"); V.tensor_add(den1, aP1, bP2)
        dd = pt("dd"); G.tensor_sub(dd, P1, Dm)
        r1_ = pt("r1_"); V.reciprocal_approx_fast(r1_, den1)
        rdd = pt("rdd"); V.reciprocal_approx_fast(rdd, dd)
        vpW = pt("vpW"); G.tensor_mul(vpW, vp, W_)
        veW = pt("veW"); V.tensor_mul(veW, ve, W_)
        t4 = pt("t4"); G.tensor_mul(t4, vpW, al)
        a1t = pt("a1t"); V.tensor_mul(a1t, t4, r1_)
        SCL.activation(lna1, a1t, AF.Ln, bias=0.0, scale=1.0)
        t5 = pt("t5"); V.tensor_mul(t5, vpW, bt)
        a2t = pt("a2t"); G.tensor_mul(a2t, t5, r1_)
        SCL.activation(lna2, a2t, AF.Ln, bias=0.0, scale=1.0)
        ut = pt("ut"); G.tensor_mul(ut, veW, rdd)
        SCL.activation(lnu, ut, AF.Ln, bias=0.0, scale=1.0)

        # rows [10, HALF]: row h*5+v <- out5[h*64+s, v*100+c] at col s*100+c
        # One 3D DMA per value, issued as each value completes.
        rows_t = rows.tile([10, HALF], f32r, tag="rows", name="rows")
        for v in range(5):
            for h in range(2):
                nc.sync.dma_start(
                    out=rows_t[h * 5 + v: h * 5 + v + 1, :],
                    in_=out5[h * 64:(h + 1) * 64, v * PC:(v + 1) * PC])

        # ---------------- main loop ----------------
        n_sc = (HALF + SC - 1) // SC     # 7 (6x1024 + 1x256)
        for isc in range(n_sc):
            base = isc * SC
            cols = min(SC, HALF - base)
            th_ps = thps.tile([128, SC], f32, tag="th_ps", name=f"th_ps{isc}")
            for o in range(0, cols, 512):
                w = min(512, cols - o)
                nc.tensor.matmul(
                    th_ps[:, o:o + w], lhsT1,
                    rows_t[0:10, base + o:base + o + w],
                    start=True, stop=True)
            bas = baspool.tile([128, SC], f32r, tag="bas", name=f"bas{isc}")
            SCL.activation(bas[:, :cols], th_ps[:, :cols], AF.Exp,
                           bias=0.0, scale=1.0)
            p_ps = pps.tile([100, SC], f32, tag="p_ps", name=f"p_ps{isc}")
            for o in range(0, cols, 512):
                w = min(512, cols - o)
                nc.tensor.matmul(
                    p_ps[:, o:o + w], lhsT2,
                    bas[:, o:o + w],
                    start=True, stop=True)
            rt = rpool.tile([100, SC], f32, tag="rt", name=f"rt{isc}")
            V.reciprocal_approx_fast(rt[:, :cols], p_ps[:, :cols])
            out_t = opool.tile([100, SC], f16, tag="out_t", name=f"out_t{isc}")
            if isc % 2 == 0:
                SCL.activation(out_t[:, :cols], rt[:, :cols], AF.Copy,
                               bias=float(K1p), scale=1.0)
            else:
                G.tensor_scalar_add(out_t[:, :cols], rt[:, :cols], float(K1p))
            for h in range(2):
                nc.sync.dma_start(
                    out=sig[:, h * HALF + base: h * HALF + base + cols],
                    in_=out_t[h * TS:(h + 1) * TS, :cols])

    nc.compile()
    return nc


def _host_prep(param: np.ndarray, sample_time: np.ndarray, Cp: np.ndarray):
    """AIF conv matrix + per-channel exponential-sum fits -> lhsT1/lhsT2."""
    f32 = np.float32
    t32 = np.arange(LF, dtype=f32) * f32(STEP)
    aifci = np.interp(
        t32.astype(np.float64),
        np.asarray(sample_time, np.float64),
        np.asarray(Cp, np.float64))
    aif = np.concatenate([np.zeros(DELAY), aifci[:-DELAY]])
    idx = np.minimum(
        np.searchsorted(t32, np.asarray(sample_time, f32), side="left"),
        LF - 1)
    A = np.zeros((TS, LF))
    for k in range(TS):
        i = int(idx[k])
        A[k, : i + 1] = aif[i::-1]

    # empirical theta ranges (cheap fp64 host pass over the param maps)
    ve, vp, fp_, ps_ = [np.asarray(param[i], np.float64).ravel()
                        for i in range(4)]
    Te = ve / ps_
    S = (vp + ve) / fp_ + Te
    TcTe = (vp / fp_) * Te
    disc = np.sqrt(S * S - 4.0 * TcTe)
    thm = 2.0 / (S + disc)
    thp = (S + disc) / (2.0 * TcTe)
    tm = (float(thm.min()) * 0.98, float(thm.max()) * 1.02)
    tp = (float(thp.min()) * 0.98, float(thp.max()) * 1.02)

    def fit(nodes, tlo, thi, ngrid=3000, lam_rel=1e-10):
        th = np.geomspace(tlo, thi, ngrid)
        Eg = np.exp(-STEP * np.outer(th, np.arange(LF)))
        F = Eg @ A.T
        B = np.exp(-STEP * np.outer(th, nodes))
        lam = lam_rel * np.linalg.norm(B, 2) ** 2
        return np.linalg.solve(B.T @ B + lam * np.eye(len(nodes)), B.T @ F)

    nm16 = np.concatenate([[0.0], np.geomspace(0.5, 588.0, 15)])
    np16 = np.concatenate([[0.0], np.geomspace(0.1, 588.0, 15)])
    np15 = np.concatenate([[0.0], np.geomspace(0.1, 588.0, 14)])
    Cm = fit(nm16, *tm)      # [16, TS]
    Cq = fit(np16, *tp)      # [16, TS]
    Cq15 = fit(np15, *tp)    # [15, TS]

    K1p, K2p, VH0, SS = _spgr_consts()
    q = SS / K2p

    wall = np.zeros((128, PC + 128), f32)
    w2 = wall[:, 0:PC]
    w1 = wall[0:10, PC:PC + 128]
    for h in range(2):
        b = h * 64
        k0 = h * TS
        w2[b + 0:b + 16, k0:k0 + TS] = (q * Cm).astype(f32)
        w2[b + 16:b + 32, k0:k0 + TS] = (q * Cq).astype(f32)
        w2[b + 32:b + 48, k0:k0 + TS] = (q * Cm).astype(f32)
        w2[b + 48:b + 63, k0:k0 + TS] = (-q * Cq15).astype(f32)
        w2[b + 63, k0:k0 + TS] = f32(VH0 / K2p)
        r0 = h * 5
        w1[r0 + 0, b + 0:b + 16] = (-0.2 * nm16).astype(f32)
        w1[r0 + 0, b + 32:b + 48] = (-0.2 * nm16).astype(f32)
        w1[r0 + 1, b + 16:b + 32] = (-0.05 * np16).astype(f32)
        w1[r0 + 1, b + 48:b + 63] = (-0.05 * np15).astype(f32)
        w1[r0 + 2, b + 0:b + 16] = 1.0
        w1[r0 + 3, b + 16:b + 32] = 1.0
        w1[r0 + 4, b + 32:b + 63] = 1.0
    return wall


def kernel(param: np.ndarray, sample_time: np.ndarray, Cp: np.ndarray) -> np.ndarray:
    from concourse.bass_utils import run_bass_kernel_spmd

    if "nc" not in _CACHE:
        _CACHE["nc"] = _build_bass()
    nc = _CACHE["nc"]

    wall = _host_prep(param, sample_time, Cp)
    pflat = np.ascontiguousarray(np.asarray(param, np.float32).reshape(4, NPIX))
    in_maps = []
    for c in range(NCORES):
        in_maps.append({
            "pmap": np.ascontiguousarray(pflat[:, c * SHARD:(c + 1) * SHARD]),
            "wall": wall,
        })
    ncr = int(os.environ.get("DCE_CORES", str(NCORES)))
    res = run_bass_kernel_spmd(
        nc, in_maps[:ncr], core_ids=list(range(ncr)),
        trace=bool(int(os.environ.get("DCE_TRACE", "0"))),
    )
    if res.exec_time_ns is not None:
        _CACHE["exec_time_ns"] = res.exec_time_ns
    outs = [r["sig"] for r in res.results]
    while len(outs) < NCORES:
        outs.append(np.zeros((TS, SHARD), np.float16))
    out = np.concatenate(outs, axis=1).astype(np.float32)
    return out.reshape(TS, 1, H, W)


# revision 35
# speedup vs baseline: 1.5091x; 1.0252x over previous
"""Trainium2 Bass kernel for the DCE 2CXM signal model — log-folded 4-channel
exp basis, reciprocal epilogue.

Math per pixel: theta_m/theta_p from the 2CXM params; conc[k] =
a1*Gk(thm) + a2*Gk(thp) + u*(Gk(thm) - Gk(thp)) with a1, a2, u all > 0
(a1 = vp*alpha/(alpha*Sm+beta*Sp), a2 = vp*beta/(...), u = ve/(Sm-Sp)).
Gk(th) = sum_t A[k,t] exp(-0.1 t th) is fitted per channel with 16 (or 15)
exponential nodes on the channel's empirical theta range.  The positive
coefficients are folded into the exponentials via logs:
c*exp(-0.1 s th) = exp(-0.1 s th + ln c), so one broadcast matmul (mm1,
K=10) builds all 128 exp arguments per column (2 pixels/column, 64
partitions each: 16 a1|m + 16 a2|p + 16 u|m + 15 u|p + 1 zero -> exp=1),
one ACT Exp evaluates the basis, and one block-diagonal matmul (mm2,
K=128 -> M=100) contracts straight to P' = (VH0 + s*conc)/K2p for both
pixel halves at once (the exp(0)=1 row carries the VH0 bias).  The SPGR
epilogue uses coth(v) ~= 1/v (abs err < 0.04 on a ~300 signal):
sig = K1p + K2p/v, i.e. one DVE reciprocal + one add (+fp16 cast) spread
over ACT/Pool.  Output leaves the device in fp16; host adds K1p during
the fp32 upcast.
"""

import os
from contextlib import ExitStack

import numpy as np

H = W = 320
NPIX = H * W
NCORES = 8
SHARD = NPIX // NCORES      # 12800 pixels per core
HALF = SHARD // 2           # 6400   (2 pixels per basis column)
PC = 100                    # prep layout [128, 100]
TS = 50
STEP = 0.1
DELAY = 30
LF = 589
SC = 1024                   # superchunk columns

SIG_BASELINE = 100.0
R1 = 1.0
R1CA = 4.3
FA = 10.0
TR = 0.00487

_CACHE: dict = {}


def _spgr_consts():
    f32 = np.float32
    fa = FA * np.pi / 180.0
    cosf = float(np.cos(f32(fa)))
    sinf = float(np.sin(f32(fa)))
    E1 = float(np.exp(f32(-TR * R1)))
    M0 = SIG_BASELINE * (1.0 - cosf * E1) / (sinf * (1.0 - E1))
    M0t = M0 * sinf
    M_st = M0t * (1.0 - E1) / (1.0 - E1 * cosf)
    C0 = SIG_BASELINE - M_st
    K1 = C0 + M0t / cosf
    K2 = M0t * (cosf - 1.0) / cosf
    K1p = K1 + K2 / 2.0
    K2p = -K2 / 2.0
    VH0 = 0.5 * (-TR * R1 + np.log(cosf))
    SS = -TR * R1CA / 2.0
    return K1p, K2p, VH0, SS


def _patch_act_tables():
    """Make Exp/Ln/Copy resolve only to natural_log_exp_and_others so the
    table-load pass emits a single load instead of ping-ponging between
    exp_and_others and natural_log_exp_and_others (1.3us per switch)."""
    import concourse.bacc as bacc_mod
    from concourse import mybir
    from concourse.hw_specs import get_activation_tables as _orig

    AF = mybir.ActivationFunctionType
    mine = {AF.Exp, AF.Ln, AF.Copy, AF.Identity}

    def patched(arch):
        tabs = _orig(arch)
        out = {}
        for name, fns in tabs.items():
            if name == "natural_log_exp_and_others":
                out[name] = set(fns) | {AF.Copy, AF.Identity}
            else:
                out[name] = set(fns) - mine
        return out

    bacc_mod.get_activation_tables = patched


def _build_bass():
    import concourse.bass as bass
    import concourse.tile as tile
    from concourse import bacc, mybir

    _patch_act_tables()

    f32 = mybir.dt.float32
    f32r = mybir.dt.float32r
    f16 = mybir.dt.float16
    AF = mybir.ActivationFunctionType
    ALU = mybir.AluOpType

    nc = bacc.Bacc()
    pmap = nc.dram_tensor("pmap", [4, SHARD], f32, kind="ExternalInput")
    wall = nc.dram_tensor("wall", [128, PC + 128], f32, kind="ExternalInput")
    sig = nc.dram_tensor("sig", [TS, SHARD], f16, kind="ExternalOutput")

    K1p, K2p, VH0, SS = _spgr_consts()
    NWARM = int(os.environ.get("DCE_WARM", "0"))

    with tile.TileContext(nc) as tc, ExitStack() as ctx:
        const = ctx.enter_context(tc.tile_pool(name="const", bufs=1))
        thps = ctx.enter_context(
            tc.tile_pool(name="thps", bufs=2, space=bass.MemorySpace.PSUM))
        pps = ctx.enter_context(
            tc.tile_pool(name="pps", bufs=2, space=bass.MemorySpace.PSUM))
        baspool = ctx.enter_context(tc.tile_pool(name="bas", bufs=2))
        rpool = ctx.enter_context(tc.tile_pool(name="rp", bufs=3))
        opool = ctx.enter_context(tc.tile_pool(name="op", bufs=3))
        rows = ctx.enter_context(tc.tile_pool(name="rows", bufs=1))
        prep = ctx.enter_context(tc.tile_pool(name="prep", bufs=1))

        V = nc.vector
        G = nc.gpsimd
        SCL = nc.scalar

        # Tiny memset+exp first: starts the (single) ACT table load for
        # natural_log_exp at t~0, overlapped with the input DMAs.
        tl = const.tile([1, 1], f32, tag="tl", name="tl")
        V.memset(tl, 0.0)
        SCL.activation(tl, tl, AF.Exp, bias=0.0, scale=1.0)

        # input DMAs first (they gate the prep chain); weights after
        pin = prep.tile([128, 4 * PC], f32, tag="pin", name="pin")
        nc.sync.dma_start(
            out=pin[:, 2 * PC:4 * PC].rearrange("p (i c) -> p i c", i=2),
            in_=pmap[2:4, :].rearrange("i (p c) -> p i c", p=128))
        nc.sync.dma_start(
            out=pin[:, 0:2 * PC].rearrange("p (i c) -> p i c", i=2),
            in_=pmap[0:2, :].rearrange("i (p c) -> p i c", p=128))
        ve = pin[:, 0 * PC:1 * PC]
        vp = pin[:, 1 * PC:2 * PC]
        fp_ = pin[:, 2 * PC:3 * PC]
        ps_ = pin[:, 3 * PC:4 * PC]

        wtile_f = const.tile([128, PC + 128], f32, tag="wallf", name="wallf")
        nc.sync.dma_start(out=wtile_f, in_=wall[:])
        wtile = const.tile([128, PC + 128], f32r, tag="wallr", name="wallr")
        V.tensor_copy(wtile, wtile_f)
        lhsT2 = wtile[:, 0:PC]
        lhsT1 = wtile[0:10, PC:PC + 128]

        # PE warm-up: garbage matmuls during prep so the HAM ramp (~3us)
        # completes before the real main-loop matmuls.
        for wi in range(NWARM):
            wt = thps.tile([128, SC], f32, tag="th_ps", name=f"warm{wi}")
            nc.tensor.matmul(wt[0:PC, 0:PC], lhsT2, lhsT2[:, 0:PC],
                             start=True, stop=True)

        # ---------------- prep: pixel-major [128, 100] ----------------
        def pt(tag, dt=f32):
            return prep.tile([128, PC], dt, tag=tag, name=tag)

        out5 = prep.tile([128, 5 * PC], f32r, tag="out5", name="out5")
        thm_h = out5[:, 0 * PC:1 * PC]
        thp_h = out5[:, 1 * PC:2 * PC]
        lna1 = out5[:, 2 * PC:3 * PC]
        lna2 = out5[:, 3 * PC:4 * PC]
        lnu = out5[:, 4 * PC:5 * PC]

        rfp = pt("rfp"); V.reciprocal_approx_fast(rfp, fp_)
        rps = pt("rps"); V.reciprocal_approx_fast(rps, ps_)
        Te = pt("Te"); G.tensor_mul(Te, ve, rps)
        svp = pt("svp"); G.tensor_add(svp, vp, ve)
        T_ = pt("T_"); V.tensor_mul(T_, svp, rfp)
        Tc = pt("Tc"); G.tensor_mul(Tc, vp, rfp)
        S_ = pt("S_"); V.tensor_add(S_, T_, Te)
        TcTe = pt("TcTe"); G.tensor_mul(TcTe, Tc, Te)
        S2 = pt("S2")
        V.scalar_tensor_tensor(S2, S_, 1.0, S_, op0=ALU.mult, op1=ALU.mult)
        m4 = pt("m4"); G.tensor_scalar_mul(m4, TcTe, 4.0)
        d2 = pt("d2"); G.tensor_sub(d2, S2, m4)
        lnd = pt("lnd"); SCL.activation(lnd, d2, AF.Ln, bias=0.0, scale=1.0)
        disc = pt("disc")
        SCL.activation(disc, lnd, AF.Exp, bias=0.0, scale=0.5)
        den_ = pt("den"); G.tensor_add(den_, S_, disc)
        thmt = pt("thmt"); V.reciprocal_approx_fast(thmt, den_)
        rTT = pt("rTT"); V.reciprocal_approx_fast(rTT, TcTe)
        thpt = pt("thpt"); G.tensor_mul(thpt, den_, rTT)
        # off-chain f32r copies feeding the rows gather only
        G.tensor_copy(thm_h, thmt)
        V.tensor_copy(thp_h, thpt)

        thm_r = thmt
        thp_r = thpt
        r1m = pt("r1m")
        SCL.activation(r1m, thm_r, AF.Exp, bias=0.0, scale=-0.2)
        rlm = pt("rlm")
        SCL.activation(rlm, thm_r, AF.Exp, bias=0.0, scale=-0.2 * LF)
        r1p = pt("r1p")
        SCL.activation(r1p, thp_r, AF.Exp, bias=0.0, scale=-0.05)

        Dm = pt("Dm"); V.tensor_scalar(Dm, r1m, -1.0, 1.0, op0=ALU.mult, op1=ALU.add)
        Dp = pt("Dp"); G.tensor_scalar(Dp, r1p, -1.0, 1.0, op0=ALU.mult, op1=ALU.add)
        Nm = pt("Nm"); V.tensor_scalar(Nm, rlm, -1.0, 1.0, op0=ALU.mult, op1=ALU.add)
        P1 = pt("P1"); V.tensor_mul(P1, Nm, Dp)
        W_ = pt("W_"); G.tensor_mul(W_, Dm, Dp)
        alt = pt("alt")
        V.scalar_tensor_tensor(alt, Te, -2.0, thm_r, op0=ALU.mult, op1=ALU.mult)
        al = pt("al"); G.tensor_scalar_add(al, alt, 1.0)
        btt = pt("btt")
        V.scalar_tensor_tensor(btt, Te, 0.5, thp_r, op0=ALU.mult, op1=ALU.mult)
        bt = pt("bt"); G.tensor_scalar_sub(bt, btt, 1.0)
        aP1 = pt("aP1"); V.tensor_mul(aP1, al, P1)
        bP2 = pt("bP2"); G.tensor_mul(bP2, bt, Dm)
        den1 = pt("den1"); V.tensor_add(den1, aP1, bP2)
        dd = pt("dd"); G.tensor_sub(dd, P1, Dm)
        r1_ = pt("r1_"); V.reciprocal_approx_fast(r1_, den1)
        rdd = pt("rdd"); V.reciprocal_approx_fast(rdd, dd)
        vpW = pt("vpW"); G.tensor_mul(vpW, vp, W_)
        veW = pt("veW"); V.tensor_mul(veW, ve, W_)
        t4 = pt("t4"); G.tensor_mul(t4, vpW, al)
        a1t = pt("a1t"); V.tensor_mul(a1t, t4, r1_)
        SCL.activation(lna1, a1t, AF.Ln, bias=0.0, scale=1.0)
        t5 = pt("t5"); V.tensor_mul(t5, vpW, bt)
        a2t = pt("a2t"); G.tensor_mul(a2t, t5, r1_)
        SCL.activation(lna2, a2t, AF.Ln, bias=0.0, scale=1.0)
        ut = pt("ut"); G.tensor_mul(ut, veW, rdd)
        SCL.activation(lnu, ut, AF.Ln, bias=0.0, scale=1.0)

        # rows [10, HALF]: row h*5+v <- out5[h*64+s, v*100+c] at col s*100+c
        # One 3D DMA per value, issued as each value completes.
        rows_t = rows.tile([10, HALF], f32r, tag="rows", name="rows")
        for v in range(5):
            for h in range(2):
                nc.sync.dma_start(
                    out=rows_t[h * 5 + v: h * 5 + v + 1, :],
                    in_=out5[h * 64:(h + 1) * 64, v * PC:(v + 1) * PC])

        # ---------------- main loop ----------------
        n_sc = (HALF + SC - 1) // SC     # 7 (6x1024 + 1x256)
        for isc in range(n_sc):
            base = isc * SC
            cols = min(SC, HALF - base)
            th_ps = thps.tile([128, SC], f32, tag="th_ps", name=f"th_ps{isc}")
            for o in range(0, cols, 512):
                w = min(512, cols - o)
                nc.tensor.matmul(
                    th_ps[:, o:o + w], lhsT1,
                    rows_t[0:10, base + o:base + o + w],
                    start=True, stop=True)
            bas = baspool.tile([128, SC], f32r, tag="bas", name=f"bas{isc}")
            SCL.activation(bas[:, :cols], th_ps[:, :cols], AF.Exp,
                           bias=0.0, scale=1.0)
            p_ps = pps.tile([100, SC], f32, tag="p_ps", name=f"p_ps{isc}")
            for o in range(0, cols, 512):
                w = min(512, cols - o)
                nc.tensor.matmul(
                    p_ps[:, o:o + w], lhsT2,
                    bas[:, o:o + w],
                    start=True, stop=True)
            rt = rpool.tile([100, SC], f32, tag="rt", name=f"rt{isc}")
            V.reciprocal_approx_fast(rt[:, :cols], p_ps[:, :cols])
            out_t = opool.tile([100, SC], f16, tag="out_t", name=f"out_t{isc}")
            if isc % 4 == 3:
                SCL.activation(out_t[:, :cols], rt[:, :cols], AF.Copy,
                               bias=float(K1p), scale=1.0)
            else:
                G.tensor_scalar_add(out_t[:, :cols], rt[:, :cols], float(K1p))
            for h in range(2):
                nc.sync.dma_start(
                    out=sig[:, h * HALF + base: h * HALF + base + cols],
                    in_=out_t[h * TS:(h + 1) * TS, :cols])

    nc.compile()
    return nc


def _host_prep(param: np.ndarray, sample_time: np.ndarray, Cp: np.ndarray):
    """AIF conv matrix + per-channel exponential-sum fits -> lhsT1/lhsT2."""
    f32 = np.float32
    t32 = np.arange(LF, dtype=f32) * f32(STEP)
    aifci = np.interp(
        t32.astype(np.float64),
        np.asarray(sample_time, np.float64),
        np.asarray(Cp, np.float64))
    aif = np.concatenate([np.zeros(DELAY), aifci[:-DELAY]])
    idx = np.minimum(
        np.searchsorted(t32, np.asarray(sample_time, f32), side="left"),
        LF - 1)
    A = np.zeros((TS, LF))
    for k in range(TS):
        i = int(idx[k])
        A[k, : i + 1] = aif[i::-1]

    # empirical theta ranges (cheap fp64 host pass over the param maps)
    ve, vp, fp_, ps_ = [np.asarray(param[i], np.float64).ravel()
                        for i in range(4)]
    Te = ve / ps_
    S = (vp + ve) / fp_ + Te
    TcTe = (vp / fp_) * Te
    disc = np.sqrt(S * S - 4.0 * TcTe)
    thm = 2.0 / (S + disc)
    thp = (S + disc) / (2.0 * TcTe)
    tm = (float(thm.min()) * 0.98, float(thm.max()) * 1.02)
    tp = (float(thp.min()) * 0.98, float(thp.max()) * 1.02)

    def fit(nodes, tlo, thi, ngrid=3000, lam_rel=1e-10):
        th = np.geomspace(tlo, thi, ngrid)
        Eg = np.exp(-STEP * np.outer(th, np.arange(LF)))
        F = Eg @ A.T
        B = np.exp(-STEP * np.outer(th, nodes))
        lam = lam_rel * np.linalg.norm(B, 2) ** 2
        return np.linalg.solve(B.T @ B + lam * np.eye(len(nodes)), B.T @ F)

    nm16 = np.concatenate([[0.0], np.geomspace(0.5, 588.0, 15)])
    np16 = np.concatenate([[0.0], np.geomspace(0.1, 588.0, 15)])
    np15 = np.concatenate([[0.0], np.geomspace(0.1, 588.0, 14)])
    Cm = fit(nm16, *tm)      # [16, TS]
    Cq = fit(np16, *tp)      # [16, TS]
    Cq15 = fit(np15, *tp)    # [15, TS]

    K1p, K2p, VH0, SS = _spgr_consts()
    q = SS / K2p

    wall = np.zeros((128, PC + 128), f32)
    w2 = wall[:, 0:PC]
    w1 = wall[0:10, PC:PC + 128]
    for h in range(2):
        b = h * 64
        k0 = h * TS
        w2[b + 0:b + 16, k0:k0 + TS] = (q * Cm).astype(f32)
        w2[b + 16:b + 32, k0:k0 + TS] = (q * Cq).astype(f32)
        w2[b + 32:b + 48, k0:k0 + TS] = (q * Cm).astype(f32)
        w2[b + 48:b + 63, k0:k0 + TS] = (-q * Cq15).astype(f32)
        w2[b + 63, k0:k0 + TS] = f32(VH0 / K2p)
        r0 = h * 5
        w1[r0 + 0, b + 0:b + 16] = (-0.2 * nm16).astype(f32)
        w1[r0 + 0, b + 32:b + 48] = (-0.2 * nm16).astype(f32)
        w1[r0 + 1, b + 16:b + 32] = (-0.05 * np16).astype(f32)
        w1[r0 + 1, b + 48:b + 63] = (-0.05 * np15).astype(f32)
        w1[r0 + 2, b + 0:b + 16] = 1.0
        w1[r0 + 3, b + 16:b + 32] = 1.0
        w1[r0 + 4, b + 32:b + 63] = 1.0
    return wall


def kernel(param: np.ndarray, sample_time: np.ndarray, Cp: np.ndarray) -> np.ndarray:
    from concourse.bass_utils import run_bass_kernel_spmd

    if "nc" not in _CACHE:
        _CACHE["nc"] = _build_bass()
    nc = _CACHE["nc"]

    wall = _host_prep(param, sample_time, Cp)
    pflat = np.ascontiguousarray(np.asarray(param, np.float32).reshape(4, NPIX))
    in_maps = []
    for c in range(NCORES):
        in_maps.append({
            "pmap": np.ascontiguousarray(pflat[:, c * SHARD:(c + 1) * SHARD]),
            "wall": wall,
        })
    ncr = int(os.environ.get("DCE_CORES", str(NCORES)))
    res = run_bass_kernel_spmd(
        nc, in_maps[:ncr], core_ids=list(range(ncr)),
        trace=bool(int(os.environ.get("DCE_TRACE", "0"))),
    )
    if res.exec_time_ns is not None:
        _CACHE["exec_time_ns"] = res.exec_time_ns
    outs = [r["sig"] for r in res.results]
    while len(outs) < NCORES:
        outs.append(np.zeros((TS, SHARD), np.float16))
    out = np.concatenate(outs, axis=1).astype(np.float32)
    return out.reshape(TS, 1, H, W)


# revision 39
# speedup vs baseline: 1.5641x; 1.0364x over previous
"""Trainium2 Bass kernel for the DCE 2CXM signal model — log-folded 4-channel
exp basis, reciprocal epilogue.

Math per pixel: theta_m/theta_p from the 2CXM params; conc[k] =
a1*Gk(thm) + a2*Gk(thp) + u*(Gk(thm) - Gk(thp)) with a1, a2, u all > 0
(a1 = vp*alpha/(alpha*Sm+beta*Sp), a2 = vp*beta/(...), u = ve/(Sm-Sp)).
Gk(th) = sum_t A[k,t] exp(-0.1 t th) is fitted per channel with 16 (or 15)
exponential nodes on the channel's empirical theta range.  The positive
coefficients are folded into the exponentials via logs:
c*exp(-0.1 s th) = exp(-0.1 s th + ln c), so one broadcast matmul (mm1,
K=10) builds all 128 exp arguments per column (2 pixels/column, 64
partitions each: 16 a1|m + 16 a2|p + 16 u|m + 15 u|p + 1 zero -> exp=1),
one ACT Exp evaluates the basis, and one block-diagonal matmul (mm2,
K=128 -> M=100) contracts straight to P' = (VH0 + s*conc)/K2p for both
pixel halves at once (the exp(0)=1 row carries the VH0 bias).  The SPGR
epilogue uses coth(v) ~= 1/v (abs err < 0.04 on a ~300 signal):
sig = K1p + K2p/v, i.e. one DVE reciprocal + one add (+fp16 cast) spread
over ACT/Pool.  Output leaves the device in fp16; host adds K1p during
the fp32 upcast.
"""

import os
from contextlib import ExitStack

import numpy as np

H = W = 320
NPIX = H * W
NCORES = 8
SHARD = NPIX // NCORES      # 12800 pixels per core
HALF = SHARD // 2           # 6400   (2 pixels per basis column)
PC = 100                    # prep layout [128, 100]
TS = 50
STEP = 0.1
DELAY = 30
LF = 589
SC = 1024                   # superchunk columns

SIG_BASELINE = 100.0
R1 = 1.0
R1CA = 4.3
FA = 10.0
TR = 0.00487

_CACHE: dict = {}


def _spgr_consts():
    f32 = np.float32
    fa = FA * np.pi / 180.0
    cosf = float(np.cos(f32(fa)))
    sinf = float(np.sin(f32(fa)))
    E1 = float(np.exp(f32(-TR * R1)))
    M0 = SIG_BASELINE * (1.0 - cosf * E1) / (sinf * (1.0 - E1))
    M0t = M0 * sinf
    M_st = M0t * (1.0 - E1) / (1.0 - E1 * cosf)
    C0 = SIG_BASELINE - M_st
    K1 = C0 + M0t / cosf
    K2 = M0t * (cosf - 1.0) / cosf
    K1p = K1 + K2 / 2.0
    K2p = -K2 / 2.0
    VH0 = 0.5 * (-TR * R1 + np.log(cosf))
    SS = -TR * R1CA / 2.0
    return K1p, K2p, VH0, SS


def _patch_act_tables():
    """Make Exp/Ln/Copy resolve only to natural_log_exp_and_others so the
    table-load pass emits a single load instead of ping-ponging between
    exp_and_others and natural_log_exp_and_others (1.3us per switch)."""
    import concourse.bacc as bacc_mod
    from concourse import mybir
    from concourse.hw_specs import get_activation_tables as _orig

    AF = mybir.ActivationFunctionType
    mine = {AF.Exp, AF.Ln, AF.Copy, AF.Identity}

    def patched(arch):
        tabs = _orig(arch)
        out = {}
        for name, fns in tabs.items():
            if name == "natural_log_exp_and_others":
                out[name] = set(fns) | {AF.Copy, AF.Identity}
            else:
                out[name] = set(fns) - mine
        return out

    bacc_mod.get_activation_tables = patched


def _build_bass():
    import concourse.bass as bass
    import concourse.tile as tile
    from concourse import bacc, mybir

    _patch_act_tables()

    f32 = mybir.dt.float32
    f32r = mybir.dt.float32r
    f16 = mybir.dt.float16
    AF = mybir.ActivationFunctionType
    ALU = mybir.AluOpType

    nc = bacc.Bacc()
    pmap = nc.dram_tensor("pmap", [4, SHARD], f32, kind="ExternalInput")
    wall = nc.dram_tensor("wall", [128, PC + 128], f32, kind="ExternalInput")
    sig = nc.dram_tensor("sig", [TS, SHARD], f16, kind="ExternalOutput")

    K1p, K2p, VH0, SS = _spgr_consts()
    NWARM = int(os.environ.get("DCE_WARM", "0"))

    with tile.TileContext(nc) as tc, ExitStack() as ctx:
        const = ctx.enter_context(tc.tile_pool(name="const", bufs=1))
        thps = ctx.enter_context(
            tc.tile_pool(name="thps", bufs=2, space=bass.MemorySpace.PSUM))
        pps = ctx.enter_context(
            tc.tile_pool(name="pps", bufs=2, space=bass.MemorySpace.PSUM))
        baspool = ctx.enter_context(tc.tile_pool(name="bas", bufs=2))
        rpool = ctx.enter_context(tc.tile_pool(name="rp", bufs=3))
        opool = ctx.enter_context(tc.tile_pool(name="op", bufs=3))
        rows = ctx.enter_context(tc.tile_pool(name="rows", bufs=1))
        prep = ctx.enter_context(tc.tile_pool(name="prep", bufs=1))

        V = nc.vector
        G = nc.gpsimd
        SCL = nc.scalar

        # Tiny memset+exp first: starts the (single) ACT table load for
        # natural_log_exp at t~0, overlapped with the input DMAs.
        tl = const.tile([1, 1], f32, tag="tl", name="tl")
        V.memset(tl, 0.0)
        SCL.activation(tl, tl, AF.Exp, bias=0.0, scale=1.0)

        # input DMAs first (they gate the prep chain); weights after
        pin = prep.tile([128, 4 * PC], f32, tag="pin", name="pin")
        nc.sync.dma_start(
            out=pin[:, 2 * PC:4 * PC].rearrange("p (i c) -> p i c", i=2),
            in_=pmap[2:4, :].rearrange("i (p c) -> p i c", p=128))
        nc.sync.dma_start(
            out=pin[:, 0:2 * PC].rearrange("p (i c) -> p i c", i=2),
            in_=pmap[0:2, :].rearrange("i (p c) -> p i c", p=128))
        ve = pin[:, 0 * PC:1 * PC]
        vp = pin[:, 1 * PC:2 * PC]
        fp_ = pin[:, 2 * PC:3 * PC]
        ps_ = pin[:, 3 * PC:4 * PC]

        wtile_f = const.tile([128, PC + 128], f32, tag="wallf", name="wallf")
        nc.sync.dma_start(out=wtile_f, in_=wall[:])
        wtile = const.tile([128, PC + 128], f32r, tag="wallr", name="wallr")
        V.tensor_copy(wtile, wtile_f)
        lhsT2 = wtile[:, 0:PC]
        lhsT1 = wtile[0:10, PC:PC + 128]

        # PE warm-up: garbage matmuls during prep so the HAM ramp (~3us)
        # completes before the real main-loop matmuls.
        for wi in range(NWARM):
            wt = thps.tile([128, SC], f32, tag="th_ps", name=f"warm{wi}")
            nc.tensor.matmul(wt[0:PC, 0:PC], lhsT2, lhsT2[:, 0:PC],
                             start=True, stop=True)

        # ---------------- prep: pixel-major [128, 100] ----------------
        def pt(tag, dt=f32):
            return prep.tile([128, PC], dt, tag=tag, name=tag)

        out5 = prep.tile([128, 5 * PC], f32r, tag="out5", name="out5")
        thm_h = out5[:, 0 * PC:1 * PC]
        thp_h = out5[:, 1 * PC:2 * PC]
        lna1 = out5[:, 2 * PC:3 * PC]
        lna2 = out5[:, 3 * PC:4 * PC]
        lnu = out5[:, 4 * PC:5 * PC]

        # Critical chain lives on DVE; Pool handles off-chain branches; ACT
        # only for ln/exp. Cross-engine hops cost ~0.5us, same-engine ~0.25.
        rfp = pt("rfp"); V.reciprocal_approx_fast(rfp, fp_)
        rps = pt("rps"); V.reciprocal_approx_fast(rps, ps_)
        Te = pt("Te"); V.tensor_mul(Te, ve, rps)
        Tc = pt("Tc"); V.tensor_mul(Tc, vp, rfp)
        svp = pt("svp"); G.tensor_add(svp, vp, ve)       # off-chain (Pool)
        T_ = pt("T_"); V.tensor_mul(T_, svp, rfp)
        S_ = pt("S_"); V.tensor_add(S_, T_, Te)
        TcTe = pt("TcTe"); V.tensor_mul(TcTe, Tc, Te)
        S2 = pt("S2")
        V.scalar_tensor_tensor(S2, S_, 1.0, S_, op0=ALU.mult, op1=ALU.mult)
        d2 = pt("d2")
        V.scalar_tensor_tensor(d2, TcTe, -4.0, S2, op0=ALU.mult, op1=ALU.add)
        lnd = pt("lnd"); SCL.activation(lnd, d2, AF.Ln, bias=0.0, scale=1.0)
        disc = pt("disc")
        SCL.activation(disc, lnd, AF.Exp, bias=0.0, scale=0.5)
        # during the ACT detour, DVE computes rTT (needed right after)
        rTT = pt("rTT"); V.reciprocal_approx_fast(rTT, TcTe)
        den_ = pt("den"); V.tensor_add(den_, S_, disc)
        thmt = pt("thmt"); V.reciprocal_approx_fast(thmt, den_)
        thpt = pt("thpt"); V.tensor_mul(thpt, den_, rTT)

        r1m = pt("r1m")
        SCL.activation(r1m, thmt, AF.Exp, bias=0.0, scale=-0.2)
        r1p = pt("r1p")
        SCL.activation(r1p, thpt, AF.Exp, bias=0.0, scale=-0.05)
        rlm = pt("rlm")
        SCL.activation(rlm, thmt, AF.Exp, bias=0.0, scale=-0.2 * LF)

        # off-chain (Pool) while ACT runs: alpha/beta pieces and f32r copies
        alt = pt("alt")
        G.tensor_scalar_mul(alt, Te, -2.0)
        al0 = pt("al0"); G.tensor_mul(al0, alt, thmt)
        al = pt("al"); G.tensor_scalar_add(al, al0, 1.0)
        btt = pt("btt"); G.tensor_scalar_mul(btt, Te, 0.5)
        bt0 = pt("bt0"); G.tensor_mul(bt0, btt, thpt)
        bt = pt("bt"); G.tensor_scalar_sub(bt, bt0, 1.0)
        G.tensor_copy(thm_h, thmt)
        G.tensor_copy(thp_h, thpt)

        # DVE run 2: finish u first (so its rows DMAs launch earliest),
        # then a2, then c1-side.
        Dm = pt("Dm"); V.tensor_scalar(Dm, r1m, -1.0, 1.0, op0=ALU.mult, op1=ALU.add)
        Dp = pt("Dp"); V.tensor_scalar(Dp, r1p, -1.0, 1.0, op0=ALU.mult, op1=ALU.add)
        Nm = pt("Nm"); V.tensor_scalar(Nm, rlm, -1.0, 1.0, op0=ALU.mult, op1=ALU.add)
        P1 = pt("P1"); V.tensor_mul(P1, Nm, Dp)
        dd = pt("dd"); V.tensor_sub(dd, P1, Dm)
        rdd = pt("rdd"); V.reciprocal_approx_fast(rdd, dd)
        W_ = pt("W_"); G.tensor_mul(W_, Dm, Dp)           # Pool, off-chain
        veW = pt("veW"); G.tensor_mul(veW, ve, W_)        # Pool
        vpW = pt("vpW"); G.tensor_mul(vpW, vp, W_)        # Pool
        ut = pt("ut"); V.tensor_mul(ut, veW, rdd)
        SCL.activation(lnu, ut, AF.Ln, bias=0.0, scale=1.0)
        aP1 = pt("aP1"); V.tensor_mul(aP1, al, P1)
        bP2 = pt("bP2"); V.tensor_mul(bP2, bt, Dm)
        den1 = pt("den1"); V.tensor_add(den1, aP1, bP2)
        r1_ = pt("r1_"); V.reciprocal_approx_fast(r1_, den1)
        t5 = pt("t5"); G.tensor_mul(t5, vpW, bt)          # Pool, ready early
        a2t = pt("a2t"); V.tensor_mul(a2t, t5, r1_)
        SCL.activation(lna2, a2t, AF.Ln, bias=0.0, scale=1.0)
        t4 = pt("t4"); G.tensor_mul(t4, vpW, al)          # Pool, ready early
        a1t = pt("a1t"); V.tensor_mul(a1t, t4, r1_)
        SCL.activation(lna1, a1t, AF.Ln, bias=0.0, scale=1.0)

        # rows [10, HALF]: partitions 0:4 = theta block (thmA, thpA, thmB,
        # thpB), 4:10 = log block (lnc1A, lna2A, lnuA, lnc1B, lna2B, lnuB).
        # Issue order: theta pairs first (ready early), then lnu, lna2, lna1.
        rows_t = rows.tile([10, HALF], f32r, tag="rows", name="rows")
        ROWMAP = {(0, 0): 0, (1, 0): 1, (0, 1): 2, (1, 1): 3,
                  (2, 0): 4, (3, 0): 5, (4, 0): 6,
                  (2, 1): 7, (3, 1): 8, (4, 1): 9}
        for v in (0, 1, 4, 3, 2):
            for h in range(2):
                r = ROWMAP[(v, h)]
                nc.sync.dma_start(
                    out=rows_t[r: r + 1, :],
                    in_=out5[h * 64:(h + 1) * 64, v * PC:(v + 1) * PC])

        # ---------------- main loop ----------------
        n_sc = (HALF + SC - 1) // SC     # 7 (6x1024 + 1x256)
        for isc in range(n_sc):
            base = isc * SC
            cols = min(SC, HALF - base)
            th_ps = thps.tile([128, SC], f32, tag="th_ps", name=f"th_ps{isc}")
            for o in range(0, cols, 512):
                w = min(512, cols - o)
                nc.tensor.matmul(
                    th_ps[:, o:o + w], lhsT1,
                    rows_t[0:10, base + o:base + o + w],
                    start=True, stop=True)
            bas = baspool.tile([128, SC], f32r, tag="bas", name=f"bas{isc}")
            SCL.activation(bas[:, :cols], th_ps[:, :cols], AF.Exp,
                           bias=0.0, scale=1.0)
            p_ps = pps.tile([100, SC], f32, tag="p_ps", name=f"p_ps{isc}")
            for o in range(0, cols, 512):
                w = min(512, cols - o)
                nc.tensor.matmul(
                    p_ps[:, o:o + w], lhsT2,
                    bas[:, o:o + w],
                    start=True, stop=True)
            rt = rpool.tile([100, SC], f32, tag="rt", name=f"rt{isc}")
            V.reciprocal_approx_fast(rt[:, :cols], p_ps[:, :cols])
            out_t = opool.tile([100, SC], f16, tag="out_t", name=f"out_t{isc}")
            if isc in (1, 4, 6):
                SCL.activation(out_t[:, :cols], rt[:, :cols], AF.Copy,
                               bias=float(K1p), scale=1.0)
            else:
                G.tensor_scalar_add(out_t[:, :cols], rt[:, :cols], float(K1p))
            for h in range(2):
                nc.sync.dma_start(
                    out=sig[:, h * HALF + base: h * HALF + base + cols],
                    in_=out_t[h * TS:(h + 1) * TS, :cols])

    nc.compile()
    return nc


def _host_prep(param: np.ndarray, sample_time: np.ndarray, Cp: np.ndarray):
    """AIF conv matrix + per-channel exponential-sum fits -> lhsT1/lhsT2."""
    f32 = np.float32
    t32 = np.arange(LF, dtype=f32) * f32(STEP)
    aifci = np.interp(
        t32.astype(np.float64),
        np.asarray(sample_time, np.float64),
        np.asarray(Cp, np.float64))
    aif = np.concatenate([np.zeros(DELAY), aifci[:-DELAY]])
    idx = np.minimum(
        np.searchsorted(t32, np.asarray(sample_time, f32), side="left"),
        LF - 1)
    A = np.zeros((TS, LF))
    for k in range(TS):
        i = int(idx[k])
        A[k, : i + 1] = aif[i::-1]

    # empirical theta ranges (cheap fp64 host pass over the param maps)
    ve, vp, fp_, ps_ = [np.asarray(param[i], np.float64).ravel()
                        for i in range(4)]
    Te = ve / ps_
    S = (vp + ve) / fp_ + Te
    TcTe = (vp / fp_) * Te
    disc = np.sqrt(S * S - 4.0 * TcTe)
    thm = 2.0 / (S + disc)
    thp = (S + disc) / (2.0 * TcTe)
    tm = (float(thm.min()) * 0.98, float(thm.max()) * 1.02)
    tp = (float(thp.min()) * 0.98, float(thp.max()) * 1.02)

    def fit(nodes, tlo, thi, ngrid=3000, lam_rel=1e-10):
        th = np.geomspace(tlo, thi, ngrid)
        Eg = np.exp(-STEP * np.outer(th, np.arange(LF)))
        F = Eg @ A.T
        B = np.exp(-STEP * np.outer(th, nodes))
        lam = lam_rel * np.linalg.norm(B, 2) ** 2
        return np.linalg.solve(B.T @ B + lam * np.eye(len(nodes)), B.T @ F)

    nm16 = np.concatenate([[0.0], np.geomspace(0.5, 588.0, 15)])
    np16 = np.concatenate([[0.0], np.geomspace(0.1, 588.0, 15)])
    np15 = np.concatenate([[0.0], np.geomspace(0.1, 588.0, 14)])
    Cm = fit(nm16, *tm)      # [16, TS]
    Cq = fit(np16, *tp)      # [16, TS]
    Cq15 = fit(np15, *tp)    # [15, TS]

    K1p, K2p, VH0, SS = _spgr_consts()
    q = SS / K2p

    wall = np.zeros((128, PC + 128), f32)
    w2 = wall[:, 0:PC]
    w1 = wall[0:10, PC:PC + 128]
    for h in range(2):
        b = h * 64
        k0 = h * TS
        w2[b + 0:b + 16, k0:k0 + TS] = (q * Cm).astype(f32)
        w2[b + 16:b + 32, k0:k0 + TS] = (q * Cq).astype(f32)
        w2[b + 32:b + 48, k0:k0 + TS] = (q * Cm).astype(f32)
        w2[b + 48:b + 63, k0:k0 + TS] = (-q * Cq15).astype(f32)
        w2[b + 63, k0:k0 + TS] = f32(VH0 / K2p)
        r0 = h * 5
        # rows layout: 0-3 = theta block (thmA, thpA, thmB, thpB),
        # 4-9 = log block (lna1A, lna2A, lnuA, lna1B, lna2B, lnuB)
        r_thm = 0 + 2 * h
        r_thp = 1 + 2 * h
        r_ln1 = 4 + 3 * h
        r_ln2 = 5 + 3 * h
        r_lnu = 6 + 3 * h
        w1[r_thm, b + 0:b + 16] = (-0.2 * nm16).astype(f32)
        w1[r_thm, b + 32:b + 48] = (-0.2 * nm16).astype(f32)
        w1[r_thp, b + 16:b + 32] = (-0.05 * np16).astype(f32)
        w1[r_thp, b + 48:b + 63] = (-0.05 * np15).astype(f32)
        w1[r_ln1, b + 0:b + 16] = 1.0
        w1[r_ln2, b + 16:b + 32] = 1.0
        w1[r_lnu, b + 32:b + 63] = 1.0
    return wall


def kernel(param: np.ndarray, sample_time: np.ndarray, Cp: np.ndarray) -> np.ndarray:
    from concourse.bass_utils import run_bass_kernel_spmd

    if "nc" not in _CACHE:
        _CACHE["nc"] = _build_bass()
    nc = _CACHE["nc"]

    wall = _host_prep(param, sample_time, Cp)
    pflat = np.ascontiguousarray(np.asarray(param, np.float32).reshape(4, NPIX))
    in_maps = []
    for c in range(NCORES):
        in_maps.append({
            "pmap": np.ascontiguousarray(pflat[:, c * SHARD:(c + 1) * SHARD]),
            "wall": wall,
        })
    ncr = int(os.environ.get("DCE_CORES", str(NCORES)))
    res = run_bass_kernel_spmd(
        nc, in_maps[:ncr], core_ids=list(range(ncr)),
        trace=bool(int(os.environ.get("DCE_TRACE", "0"))),
    )
    if res.exec_time_ns is not None:
        _CACHE["exec_time_ns"] = res.exec_time_ns
    outs = [r["sig"] for r in res.results]
    while len(outs) < NCORES:
        outs.append(np.zeros((TS, SHARD), np.float16))
    out = np.concatenate(outs, axis=1).astype(np.float32)
    return out.reshape(TS, 1, H, W)
